# revision 14
# baseline (speedup 1.0000x reference)
"""Trainium2 Bass kernel for a 2-layer GraphSAGE encoder (adversarial variant).

Computes, matching the reference:
    h   = meanagg(x) @ Wl1 + bl1 + x @ Wr1 + perturb_first
    out = meanagg(h) @ Wl2 + bl2 + h @ Wr2 + perturb_last
where meanagg is the in-edge mean aggregation (segment-mean over
edge_index[0] -> edge_index[1]).

Strategy (8 NeuronCores, graph/data parallel):
  * Nodes are sharded contiguously across the 8 cores (dst side); edges are
    assigned to the core owning their destination.
  * meanagg is computed as a sequence of tiny segment matmuls: gathered
    source rows [128 edges, 128 feat] (fp16) x one-hot selection matrices
    built on-device from per-edge local-slot ids (is_equal against an iota
    tile, 4x DVE mode), accumulated per 128-node block in PSUM (fp32) and
    scaled by 1/deg in fp32 afterwards.
  * The gather uses the SWDGE dma_gather instruction (int16 indices =>
    source table processed in 32768-row chunks; indices wrapped [16, n/16]
    and replicated across the 8 GPSIMD core stripes; 4 SWDGE queues).
  * Layer 2 is algebraically reordered: out = meanagg(h @ Wl2) + (h @ Wr2 +
    bl2 + perturb_last), so the layer-2 gather moves 128-wide rows instead
    of 256-wide ones.  Pass A computes hl = h@Wl2 and pout = h@Wr2+bl2+p2
    per shard; the host concatenates hl across shards (pure data movement)
    and pass B computes out = meanagg(hl) + pout.
  * All matmuls run in fp16 (1 PE cycle/row vs 4 for fp32): the selection
    matrices are exact 0/1, accumulation stays fp32 in PSUM, and 1/deg is
    applied in fp32 — only activations/weights are rounded to fp16.
  * Block-tiled tensors (p2/hl/po/ivn/out) use pre-tiled DRAM layouts
    ([NGRP, 128, GB*F], host-reshaped) so every DMA descriptor is a
    contiguous >=512B run.
  * All per-core programs are identical (one SPMD NEFF); per-(group,chunk,
    block) run lengths are padded to the max across cores so only the DATA
    (indices / selection values) differs per core.
"""

import sys

import numpy as np

if "/opt/trn_rl_repo" not in sys.path:
    sys.path.insert(0, "/opt/trn_rl_repo")

import concourse.bacc as bacc
import concourse.tile as tile
import concourse.mybir as mybir
from concourse.bass_utils import run_bass_kernel_spmd as _run_spmd


def run_bass_kernel_spmd(nc, in_maps, core_ids):
    """Run with retries: a previously crashed process can leave a NeuronCore
    briefly wedged; back off and retry."""
    import time as _time
    last = None
    for attempt in range(3):
        try:
            return _run_spmd(nc, in_maps, core_ids=core_ids)
        except Exception as e:  # noqa: BLE001 - device-transient errors
            last = e
            _time.sleep(15 * (attempt + 1))
    raise last

P = 128          # partitions / block size
NC = 8           # cores
GB = 4           # node blocks per group
CHSZ = 32768     # int16 gather chunk (rows)
GMAX = 1024      # max idxs per dma_gather (Q7 scratch limit)
FP = mybir.dt.float32
F16 = mybir.dt.float16
FR = mybir.dt.float32r


def _cdiv(a, b):
    return (a + b - 1) // b


def _r(ap):
    """View an fp32 access pattern as float32r (tf32-like matmul inputs)."""
    return ap.bitcast(FR)


# ----------------------------------------------------------------------------
# Host-side preprocessing: pure integer index work + 1/deg table.
# ----------------------------------------------------------------------------
class Plan:
    pass


def _preprocess(edge_index, n_nodes):
    src = np.asarray(edge_index[0]).astype(np.int64)
    dst = np.asarray(edge_index[1]).astype(np.int64)

    pl = Plan()
    pl.N = n_nodes
    pl.SH = _cdiv(n_nodes, NC)                  # shard rows
    pl.NB = _cdiv(pl.SH, P)                     # node blocks per shard
    pl.NGRP = _cdiv(pl.NB, GB)                  # block groups
    pl.SHP = pl.NGRP * GB * P                   # padded shard rows
    pl.NCH = _cdiv(n_nodes, CHSZ)               # src chunks

    deg = np.bincount(dst, minlength=n_nodes)
    invd_node = (1.0 / np.maximum(deg, 1)).astype(np.float32)

    core = dst // pl.SH
    ldst = dst - core * pl.SH
    blk = ldst >> 7
    grp = blk // GB
    bb = blk - grp * GB
    chk = src // CHSZ

    rid = ((core * pl.NGRP + grp) * pl.NCH + chk) * GB + bb
    nrun = NC * pl.NGRP * pl.NCH * GB
    counts = np.bincount(rid, minlength=nrun).reshape(NC, pl.NGRP, pl.NCH, GB)
    run_len = counts.max(axis=0)                # [NGRP, NCH, GB] shared

    off_b = np.zeros((pl.NGRP, pl.NCH, GB + 1), np.int64)
    off_b[..., 1:] = np.cumsum(run_len, axis=-1)
    tot_gc = off_b[..., -1]                     # [NGRP, NCH]
    T_gc = ((tot_gc + P - 1) // P).astype(np.int64)   # tiles per (g,ch)

    # idx column offsets (16-wrapped, (g-major, ch-minor) order)
    col16 = (T_gc * P // 16).reshape(-1)
    col16_off = np.zeros(pl.NGRP * pl.NCH + 1, np.int64)
    col16_off[1:] = np.cumsum(col16)
    pl.col16_off = col16_off
    pl.IDXCOLS = int(col16_off[-1])

    # matmul entry table, in emission order (g, bb, ch, tile)
    ecol = {}
    entries_per_block = [[[] for _ in range(GB)] for _ in range(pl.NGRP)]
    ncol = 0
    for g in range(pl.NGRP):
        for b in range(GB):
            if g * GB + b >= pl.NB:
                continue
            for ch in range(pl.NCH):
                lo = int(off_b[g, ch, b])
                hi = int(off_b[g, ch, b + 1])
                if hi == lo:
                    continue
                for t in range(lo // P, (hi - 1) // P + 1):
                    ecol[(g, ch, b, t)] = ncol
                    entries_per_block[g][b].append((ch, t, ncol))
                    ncol += 1
    pl.entries_per_block = entries_per_block
    pl.MMTOT = ncol
    mm_off = [0] * (pl.NGRP + 1)
    for g in range(pl.NGRP):
        mm_off[g + 1] = mm_off[g] + sum(
            len(entries_per_block[g][b]) for b in range(GB)
        )
    pl.mm_off = mm_off
    pl.T_gc = T_gc
    pl.Tmax_ch = [int(T_gc[:, ch].max()) for ch in range(pl.NCH)]

    # ---- per-core data arrays ----
    order = np.argsort(rid, kind="stable")
    rid_s = rid[order]
    run_start = np.zeros(nrun + 1, np.int64)
    np.cumsum(np.bincount(rid_s, minlength=nrun), out=run_start[1:])
    rank = np.arange(len(order)) - run_start[rid_s]

    e_core = core[order]
    e_g = grp[order]
    e_ch = chk[order]
    e_bb = bb[order]
    e_src16 = (src[order] - e_ch * CHSZ).astype(np.int16)
    e_l = (ldst[order] & 127).astype(np.float32)

    pos = off_b[e_g, e_ch, e_bb] + rank
    e_t = pos // P
    e_lane = pos % P

    maxT = int(T_gc.max()) if pl.MMTOT else 1
    ecol_arr = np.full((pl.NGRP, pl.NCH, GB, maxT), -1, np.int64)
    for (g, ch, b, t), c in ecol.items():
        ecol_arr[g, ch, b, t] = c
    e_col = ecol_arr[e_g, e_ch, e_bb, e_t]
    assert (e_col >= 0).all()

    IDX = np.zeros((NC, 16, pl.IDXCOLS), np.int16)
    LV = np.full((NC, P, max(pl.MMTOT, 1)), -1.0, np.float32)
    gc_flat = e_g * pl.NCH + e_ch
    idx_col = pl.col16_off[gc_flat] + pos // 16
    IDX[e_core, pos % 16, idx_col] = e_src16
    LV[e_core, e_lane, e_col] = e_l
    # two copies: each Q7 core of the handling queue pair reads its own
    # 16-partition stripe
    pl.IDX = np.tile(IDX, (1, 2, 1))
    pl.LV = LV
    pl.invd_node = invd_node
    return pl


# ----------------------------------------------------------------------------
# Shared kernel piece: gather + segment-matmul aggregation for one group.
# Produces per-block [node, feat] fp32 psum chains (un-normalized sums).
# ----------------------------------------------------------------------------
def _emit_group_aggregation(nc, pl, pools, g, src_d, idx_d, lv_d, iota_t):
    (idxp, msgp, lvp, sp, chp) = pools
    NCH = pl.NCH
    c0 = int(pl.col16_off[g * NCH])
    c1 = int(pl.col16_off[g * NCH + NCH])
    idx_t = idxp.tile([P, max(c1 - c0, 1)], mybir.dt.int16, tag="idx", name="idx")
    for ch in range(NCH):
        q = (g + ch) % 4
        a0 = int(pl.col16_off[g * NCH + ch])
        a1 = int(pl.col16_off[g * NCH + ch + 1])
        if a1 > a0:
            nc.sync.dma_start(idx_t[32 * q:32 * q + 32, a0 - c0:a1 - c0],
                              idx_d[:, a0:a1])

    m0, m1 = pl.mm_off[g], pl.mm_off[g + 1]
    mm_g = max(m1 - m0, 1)
    lv_t = lvp.tile([P, mm_g], FP, tag="lv", name="lv")
    nc.sync.dma_start(lv_t[:], lv_d[:, m0:m0 + mm_g])

    msgs = []
    for ch in range(NCH):
        T = int(pl.T_gc[g, ch])
        Tmax = pl.Tmax_ch[ch]
        msg = msgp.tile([P, max(Tmax, 1), P], F16, tag=f"msg{ch}", name=f"msg{ch}")
        base = ch * CHSZ
        rows = min(CHSZ, pl.N - base)
        gcoff = int(pl.col16_off[g * NCH + ch]) - c0
        for t0 in range(0, T, GMAX // P):
            nt = min(GMAX // P, T - t0)
            n = nt * P
            nc.gpsimd.dma_gather(
                msg[:, t0:t0 + nt, :],
                src_d[base:base + rows, :],
                idx_t[:, gcoff + t0 * (P // 16): gcoff + (t0 + nt) * (P // 16)],
                n, n, P,
                queue_num=(g + ch) % 4,
            )
        msgs.append(msg)

    chains = []
    for b in range(GB):
        if g * GB + b >= pl.NB or not pl.entries_per_block[g][b]:
            chains.append(None)
            continue
        ents = pl.entries_per_block[g][b]
        ps = chp.tile([P, P], FP, space="PSUM", tag="chain", name="chain")
        for j, (ch, t, col) in enumerate(ents):
            cl = col - m0
            s_t = sp.tile([P, P], F16, tag="s", name="s")
            nc.vector.tensor_scalar(
                out=s_t[:], in0=iota_t[:],
                scalar1=lv_t[:, cl:cl + 1], scalar2=None,
                op0=mybir.AluOpType.is_equal,
            )
            nc.tensor.matmul(ps[:], s_t[:], msgs[ch][:, t, :],
                             start=(j == 0), stop=(j == len(ents) - 1))
        chains.append(ps)
    return chains


# ----------------------------------------------------------------------------
# Pass A program: aggregation of x + both dense layers -> hl (fp16), pout
# ----------------------------------------------------------------------------
def _build_pass_a(pl, d_in, d_hid, d_out):
    assert d_in == 128 and d_hid == 256 and d_out == 128
    nc = bacc.Bacc("TRN2", target_bir_lowering=False, debug=False,
                   num_swdge_queues=4)
    x_d = nc.dram_tensor("x", [pl.N, P], F16, kind="ExternalInput").ap()
    idx_d = nc.dram_tensor("idx", [32, pl.IDXCOLS], mybir.dt.int16,
                           kind="ExternalInput").ap()
    lv_d = nc.dram_tensor("lv", [P, max(pl.MMTOT, 1)], FP, kind="ExternalInput").ap()
    iota_d = nc.dram_tensor("iota", [P, P], F16, kind="ExternalInput").ap()
    id16_d = nc.dram_tensor("id16", [P, P], F16, kind="ExternalInput").ap()
    ivn_d = nc.dram_tensor("ivn", [pl.NGRP, P, GB], FP, kind="ExternalInput").ap()
    xT_d = nc.dram_tensor("xT", [P, pl.SHP], F16, kind="ExternalInput").ap()
    p1T_d = nc.dram_tensor("p1T", [d_hid, pl.SHP], F16, kind="ExternalInput").ap()
    p2_d = nc.dram_tensor("p2", [pl.NGRP, P, GB * P], F16, kind="ExternalInput").ap()
    wl1h_d = nc.dram_tensor("wl1h", [P, d_hid], F16, kind="ExternalInput").ap()
    wr1_d = nc.dram_tensor("wr1", [P, d_hid], F16, kind="ExternalInput").ap()
    w2a_d = nc.dram_tensor("w2a", [P, 2 * d_out], F16, kind="ExternalInput").ap()
    w2b_d = nc.dram_tensor("w2b", [P, 2 * d_out], F16, kind="ExternalInput").ap()
    b1_d = nc.dram_tensor("b1", [1, d_hid], F16, kind="ExternalInput").ap()
    bc_d = nc.dram_tensor("bc", [1, 2 * d_out], F16, kind="ExternalInput").ap()
    
    hl_d = nc.dram_tensor("hl", [pl.NGRP, P, GB * P], F16, kind="ExternalOutput").ap()
    po_d = nc.dram_tensor("po", [pl.NGRP, P, GB * P], F16, kind="ExternalOutput").ap()

    with tile.TileContext(nc) as tc:
        with (
            tc.tile_pool(name="cb", bufs=1) as cb,
            tc.tile_pool(name="idxp", bufs=2) as idxp,
            tc.tile_pool(name="msgp", bufs=2) as msgp,
            tc.tile_pool(name="lvp", bufs=2) as lvp,
            tc.tile_pool(name="sp", bufs=8) as sp,
            tc.tile_pool(name="aggp", bufs=2) as aggp,
            tc.tile_pool(name="hp", bufs=2) as hp,
            tc.tile_pool(name="iop", bufs=2) as iop,
            tc.tile_pool(name="outp", bufs=2) as outp,
            tc.tile_pool(name="chp", bufs=3, space="PSUM") as chp,
            tc.tile_pool(name="trp", bufs=1, space="PSUM") as trp,
            tc.tile_pool(name="php", bufs=2, space="PSUM") as php,
            tc.tile_pool(name="pop", bufs=2, space="PSUM") as pop,
        ):
            iota_t = cb.tile([P, P], F16)
            nc.sync.dma_start(iota_t[:], iota_d[:])
            id16_t = cb.tile([P, P], F16)
            nc.sync.dma_start(id16_t[:], id16_d[:])
            wl1h_t = cb.tile([P, d_hid], F16)
            nc.sync.dma_start(wl1h_t[:], wl1h_d[:])
            wr1_t = cb.tile([P, d_hid], F16)
            nc.sync.dma_start(wr1_t[:], wr1_d[:])
            w2a_t = cb.tile([P, 2 * d_out], F16)
            nc.sync.dma_start(w2a_t[:], w2a_d[:])
            w2b_t = cb.tile([P, 2 * d_out], F16)
            nc.sync.dma_start(w2b_t[:], w2b_d[:])
            b1_t = cb.tile([1, d_hid], F16)
            nc.sync.dma_start(b1_t[:], b1_d[:])
            bc_t = cb.tile([1, 2 * d_out], F16)
            nc.sync.dma_start(bc_t[:], bc_d[:])
            ones_t = cb.tile([1, GB * P], F16)
            nc.vector.memset(ones_t[:], 1.0)

            pools = (idxp, msgp, lvp, sp, chp)
            for g in range(pl.NGRP):
                chains = _emit_group_aggregation(
                    nc, pl, pools, g, x_d, idx_d, lv_d, iota_t)
                gc0 = g * GB * P
                span = GB * P
                iv_t = iop.tile([P, GB], FP, tag="iv", name="iv")
                nc.sync.dma_start(iv_t[:], ivn_d[g])
                agg_t = aggp.tile([P, GB * P], F16, tag="aggT", name="aggT")
                for b in range(GB):
                    if chains[b] is None:
                        if g * GB + b < pl.NB:
                            nc.vector.memset(agg_t[:, b * P:(b + 1) * P], 0.0)
                        continue
                    # mean = sum * (1/deg)   (fp32 -> fp16), node-major
                    agg_nm = sp.tile([P, P], F16, tag="aggnm", name="aggnm")
                    nc.vector.tensor_scalar(
                        out=agg_nm[:], in0=chains[b][:],
                        scalar1=iv_t[:, b:b + 1], scalar2=None,
                        op0=mybir.AluOpType.mult,
                    )
                    tp = trp.tile([P, P], F16, space="PSUM", tag="tp", name="tp")
                    nc.tensor.transpose(tp[:], agg_nm[:], id16_t[:])
                    nc.any.tensor_copy(agg_t[:, b * P:(b + 1) * P], tp[:])
                # fake blocks: zero agg cols
                for b in range(GB):
                    if g * GB + b >= pl.NB:
                        nc.vector.memset(agg_t[:, b * P:(b + 1) * P], 0.0)

                xT_t = iop.tile([P, GB * P], F16, tag="xT", name="xT")
                nc.sync.dma_start(xT_t[:], xT_d[:, gc0:gc0 + span])
                p2_t = iop.tile([P, GB, P], F16, tag="p2", name="p2")
                nc.sync.dma_start(p2_t[:].rearrange("p t f -> p (t f)"), p2_d[g])
                hl_o = outp.tile([P, GB, P], F16, tag="hlo", name="hlo")
                po_o = outp.tile([P, GB, P], F16, tag="poo", name="poo")

                ph0 = php.tile([P, GB * P], FP, space="PSUM", tag="ph", name="ph")
                nc.tensor.matmul(ph0[:], b1_t[:, 0:P], ones_t[:],
                                 start=True, stop=False)
                nc.tensor.matmul(ph0[:], wl1h_t[:, 0:P], agg_t[:],
                                 start=False, stop=False)
                nc.tensor.matmul(ph0[:], wr1_t[:, 0:P], xT_t[:],
                                 start=False, stop=True)
                ph1 = php.tile([P, GB * P], FP, space="PSUM", tag="ph", name="ph")
                nc.tensor.matmul(ph1[:], b1_t[:, P:2 * P], ones_t[:],
                                 start=True, stop=False)
                nc.tensor.matmul(ph1[:], wl1h_t[:, P:2 * P], agg_t[:],
                                 start=False, stop=False)
                nc.tensor.matmul(ph1[:], wr1_t[:, P:2 * P], xT_t[:],
                                 start=False, stop=True)
                p1a = hp.tile([P, GB * P], F16, tag="p1a", name="p1a")
                nc.sync.dma_start(p1a[:], p1T_d[0:P, gc0:gc0 + span])
                p1b = hp.tile([P, GB * P], F16, tag="p1b", name="p1b")
                nc.sync.dma_start(p1b[:], p1T_d[P:2 * P, gc0:gc0 + span])
                h0 = hp.tile([P, GB * P], F16, tag="h0", name="h0")
                nc.vector.tensor_add(h0[:], ph0[:], p1a[:])
                h1 = hp.tile([P, GB * P], F16, tag="h1", name="h1")
                nc.vector.tensor_add(h1[:], ph1[:], p1b[:])

                for b in range(GB):
                    if g * GB + b >= pl.NB:
                        continue
                    pps = pop.tile([P, 2 * d_out], FP, space="PSUM",
                                   tag="pps", name="pps")
                    nc.tensor.matmul(pps[:], ones_t[:, 0:P], bc_t[:],
                                     start=True, stop=False)
                    nc.tensor.matmul(pps[:], h0[:, b * P:(b + 1) * P],
                                     w2a_t[:], start=False, stop=False)
                    nc.tensor.matmul(pps[:], h1[:, b * P:(b + 1) * P],
                                     w2b_t[:], start=False, stop=True)
                    nc.any.tensor_copy(hl_o[:, b, :], pps[:, 0:d_out])
                    nc.vector.tensor_add(po_o[:, b, :],
                                         pps[:, d_out:2 * d_out],
                                         p2_t[:, b, :])
                nc.sync.dma_start(hl_d[g], hl_o[:].rearrange("p t f -> p (t f)"))
                nc.sync.dma_start(po_d[g], po_o[:].rearrange("p t f -> p (t f)"))
    nc.compile()
    return nc


# ----------------------------------------------------------------------------
# Pass B program: aggregation of hl (fp16) + add pout -> out
# ----------------------------------------------------------------------------
def _build_pass_b(pl):
    nc = bacc.Bacc("TRN2", target_bir_lowering=False, debug=False,
                   num_swdge_queues=4)
    hlf_d = nc.dram_tensor("hlf", [pl.N, P], F16, kind="ExternalInput").ap()
    idx_d = nc.dram_tensor("idx", [32, pl.IDXCOLS], mybir.dt.int16,
                           kind="ExternalInput").ap()
    lv_d = nc.dram_tensor("lv", [P, max(pl.MMTOT, 1)], FP, kind="ExternalInput").ap()
    iota_d = nc.dram_tensor("iota", [P, P], F16, kind="ExternalInput").ap()
    ivn_d = nc.dram_tensor("ivn", [pl.NGRP, P, GB], FP, kind="ExternalInput").ap()
    po_d = nc.dram_tensor("po", [pl.NGRP, P, GB * P], F16, kind="ExternalInput").ap()
    out_d = nc.dram_tensor("out", [pl.NGRP, P, GB * P], FP, kind="ExternalOutput").ap()

    with tile.TileContext(nc) as tc:
        with (
            tc.tile_pool(name="cb", bufs=1) as cb,
            tc.tile_pool(name="idxp", bufs=2) as idxp,
            tc.tile_pool(name="msgp", bufs=2) as msgp,
            tc.tile_pool(name="lvp", bufs=2) as lvp,
            tc.tile_pool(name="sp", bufs=8) as sp,
            tc.tile_pool(name="iop", bufs=2) as iop,
            tc.tile_pool(name="outp", bufs=2) as outp,
            tc.tile_pool(name="chp", bufs=6, space="PSUM") as chp,
        ):
            iota_t = cb.tile([P, P], F16)
            nc.sync.dma_start(iota_t[:], iota_d[:])
            pools = (idxp, msgp, lvp, sp, chp)
            for g in range(pl.NGRP):
                chains = _emit_group_aggregation(
                    nc, pl, pools, g, hlf_d, idx_d, lv_d, iota_t)
                gc0 = g * GB * P
                span = GB * P
                iv_t = iop.tile([P, GB], FP, tag="iv", name="iv")
                nc.sync.dma_start(iv_t[:], ivn_d[g])
                po_t = iop.tile([P, GB, P], F16, tag="po", name="po")
                nc.sync.dma_start(po_t[:].rearrange("p t f -> p (t f)"), po_d[g])
                out_t = outp.tile([P, GB, P], FP, tag="out", name="out")
                for b in range(GB):
                    if g * GB + b >= pl.NB:
                        continue
                    if chains[b] is not None:
                        # out = sum * (1/deg) + pout, fused on DVE
                        nc.vector.scalar_tensor_tensor(
                            out=out_t[:, b, :], in0=chains[b][:],
                            scalar=iv_t[:, b:b + 1], in1=po_t[:, b, :],
                            op0=mybir.AluOpType.mult,
                            op1=mybir.AluOpType.add,
                        )
                    else:
                        nc.any.tensor_copy(out_t[:, b, :], po_t[:, b, :])
                nc.sync.dma_start(out_d[g], out_t[:].rearrange("p t f -> p (t f)"))
    nc.compile()
    return nc


# ----------------------------------------------------------------------------
# Entry point
# ----------------------------------------------------------------------------
LAST = {}


def kernel(x, edge_index, perturb_first, perturb_last,
           Wl1, bl1, Wr1, Wl2, bl2, Wr2):
    import time as _time
    x = np.ascontiguousarray(np.asarray(x, dtype=np.float32))
    n_nodes, d_in = x.shape
    d_hid = np.asarray(Wl1).shape[1]
    d_out = np.asarray(Wl2).shape[1]

    pl = _preprocess(edge_index, n_nodes)

    iota = np.tile(np.arange(P, dtype=np.float16)[None, :], (P, 1))
    id16 = np.eye(P, dtype=np.float16)
    x16 = x.astype(np.float16)
    w2cat = np.concatenate(
        [np.asarray(Wl2, np.float32), np.asarray(Wr2, np.float32)], axis=1)
    bcat = np.concatenate(
        [np.zeros(d_out, np.float32), np.asarray(bl2, np.float32)])[None, :]
    b1 = np.asarray(bl1, np.float32)[None, :]

    def shard_pad(a, rows):
        out = np.zeros((pl.SHP,) + a.shape[1:], a.dtype)
        out[: rows.stop - rows.start] = a[rows]
        return out

    def to_tiled(a):
        """[SHP, F] row-major -> [NGRP, P, GB*F] block-tiled."""
        f = a.shape[1]
        return np.ascontiguousarray(
            a.reshape(pl.NGRP, GB, P, f).transpose(0, 2, 1, 3)
            .reshape(pl.NGRP, P, GB * f))

    def from_tiled(a, f):
        """[NGRP, P, GB*F] -> [SHP, F] row-major."""
        return a.reshape(pl.NGRP, P, GB, f).transpose(0, 2, 1, 3).reshape(pl.SHP, f)

    p1 = np.asarray(perturb_first, np.float32)
    p2 = np.asarray(perturb_last, np.float32)

    in_maps_a = []
    for c in range(NC):
        rows = slice(c * pl.SH, min((c + 1) * pl.SH, n_nodes))
        xT = np.zeros((P, pl.SHP), np.float16)
        xT[:, : rows.stop - rows.start] = x16[rows].T
        p1T = np.zeros((d_hid, pl.SHP), np.float16)
        p1T[:, : rows.stop - rows.start] = p1[rows].T.astype(np.float16)
        in_maps_a.append(dict(
            x=x16, idx=pl.IDX[c], lv=pl.LV[c], iota=iota, id16=id16,
            ivn=to_tiled(shard_pad(pl.invd_node[:, None], rows)),
            xT=xT, p1T=np.ascontiguousarray(p1T),
            p2=to_tiled(shard_pad(p2.astype(np.float16), rows)),
            wl1h=np.asarray(Wl1, np.float32).astype(np.float16),
            wr1=np.asarray(Wr1, np.float32).astype(np.float16),
            w2a=np.ascontiguousarray(w2cat[0:P]).astype(np.float16),
            w2b=np.ascontiguousarray(w2cat[P:2 * P]).astype(np.float16),
            b1=b1.astype(np.float16), bc=bcat.astype(np.float16),
        ))

    nc_a = _build_pass_a(pl, d_in, d_hid, d_out)
    LAST.clear()
    LAST["nc_a"] = nc_a
    _t = _time.time()
    res_a = run_bass_kernel_spmd(nc_a, in_maps_a, core_ids=list(range(NC)))
    LAST["run_a_s"] = _time.time() - _t

    hlf = np.concatenate(
        [from_tiled(res_a.results[c]["hl"], P)[: min(pl.SH, n_nodes - c * pl.SH)]
         for c in range(NC)], axis=0)
    hlf = np.ascontiguousarray(hlf)

    in_maps_b = []
    for c in range(NC):
        rows = slice(c * pl.SH, min((c + 1) * pl.SH, n_nodes))
        in_maps_b.append(dict(
            hlf=hlf, idx=pl.IDX[c], lv=pl.LV[c], iota=iota,
            ivn=to_tiled(shard_pad(pl.invd_node[:, None], rows)),
            po=res_a.results[c]["po"],
        ))
    nc_b = _build_pass_b(pl)
    LAST["nc_b"] = nc_b
    _t = _time.time()
    res_b = run_bass_kernel_spmd(nc_b, in_maps_b, core_ids=list(range(NC)))
    LAST["run_b_s"] = _time.time() - _t

    out = np.concatenate(
        [from_tiled(res_b.results[c]["out"], P)[: min(pl.SH, n_nodes - c * pl.SH)]
         for c in range(NC)], axis=0)
    return np.ascontiguousarray(out.astype(np.float32))
